# revision 8
# baseline (speedup 1.0000x reference)
"""Grouped GEMM (8 independent fp32 matmuls, K=N=4096, varying M) on 8 TRN2 NeuronCores.

Strategy: decompose the groups into uniform row-chunks so every core gets an
identical-shape workload (SPMD): one 1280-row "big" chunk + one 768-row
"small" chunk (2048 rows/core vs 1952 ideal; 4.7% padding waste).
Each chunk is a plain matmul A_chunk @ B_group. On-device we compute
C^T tiles with the B k-tile as the stationary operand (natural layout) and
host-pre-transposed A (aT, K-major) as the moving operand, in float32r
(FP22) at full PE rate. The aT chunk stays fully SBUF-resident so each B
streams exactly once per chunk. Host side: shard + transpose A, gather +
transpose C^T back.
"""

import numpy as np

_K = 4096
_N = 4096
_KT = 32   # k tiles of 128
_NT = 32   # n tiles of 128
_R_BIG = 1280
_R_SMALL = 768
_M_LIST = [512, 1024, 2048, 3072, 4096, 1536, 2560, 768]

# Per-core chunk assignment: (group, row_start, real_rows).
_BIG_CHUNKS = [
    (4, 0, 1280), (4, 1280, 1280), (3, 0, 1280), (3, 1280, 1280),
    (6, 0, 1280), (6, 1280, 1280), (2, 0, 1280), (1, 0, 1024),
]
_SMALL_CHUNKS = [
    (4, 2560, 768), (4, 3328, 768), (3, 2560, 512), (2, 1280, 768),
    (5, 0, 768), (5, 768, 768), (7, 0, 768), (0, 0, 512),
]

_NC_CACHE = None


def _split_sync_waits(nc, limit=1):
    """walrus codegen on this toolchain rejects instructions carrying more
    than `limit` semaphore waits ("Too many sync wait commands" — notably the
    Tile kernel-tail Drain). Move excess waits onto freshly inserted
    same-engine NoOps immediately preceding the instruction: the engine
    executes its queue in order, so waits executed by the nops are
    equivalent to waits on the instruction itself."""
    import concourse.mybir as mybir

    n_split = 0
    for bb in nc.main_func.blocks:
        il = bb.instructions
        i = 0
        while i < len(il):
            ins = il[i]
            si = ins.sync_info
            if si is not None and len(si.on_wait) > limit:
                waits = list(si.on_wait)
                keep, extra = waits[-limit:], waits[:-limit]
                nops = []
                for j in range(0, len(extra), limit):
                    nop = mybir.InstNoOp(
                        name=f"{ins.name}-swsplit{j}", ins=[], outs=[]
                    )
                    nop.engine = ins.engine
                    nop.sync_info = mybir.SyncInfo(
                        on_wait=extra[j : j + limit], on_update=[]
                    )
                    nc.register_instruction(nop)
                    nops.append(nop)
                for k, nop in enumerate(nops):
                    il.insert(i + k, nop)
                ins.sync_info = mybir.SyncInfo(
                    on_wait=keep, on_update=list(si.on_update)
                )
                n_split += 1
                i += len(nops)
            i += 1
    return n_split


def build_program():
    """Build the (single, SPMD) Bass program run on each of the 8 cores."""
    global _NC_CACHE
    if _NC_CACHE is not None:
        return _NC_CACHE

    import concourse.bass as bass
    import concourse.mybir as mybir
    from concourse.tile import TileContext

    f32 = mybir.dt.float32
    f32r = mybir.dt.float32r

    nc = bass.Bass(name="group_gemm")

    at_big = nc.dram_tensor("at_big", [_K, _R_BIG], f32r, kind="ExternalInput")
    at_small = nc.dram_tensor("at_small", [_K, _R_SMALL], f32r, kind="ExternalInput")
    b_big = nc.dram_tensor("b_big", [_K, _N], f32r, kind="ExternalInput")
    b_small = nc.dram_tensor("b_small", [_K, _N], f32r, kind="ExternalInput")
    ct_big = nc.dram_tensor("ct_big", [_N, _R_BIG], f32, kind="ExternalOutput")
    ct_small = nc.dram_tensor("ct_small", [_N, _R_SMALL], f32, kind="ExternalOutput")

    chunks = [
        (at_big, b_big, ct_big, _R_BIG, (512, 512, 256)),
        (at_small, b_small, ct_small, _R_SMALL, (512, 256)),
    ]

    with TileContext(nc) as tc:
        with (
            tc.tile_pool(name="apool", bufs=1) as apool,
            tc.tile_pool(name="bpool", bufs=4) as bpool,
            tc.tile_pool(name="spool", bufs=2) as spool,
            tc.tile_pool(name="psum", bufs=6, space="PSUM") as psum_pool,
        ):
            # aT resident: 32 k-tiles x (128 partitions x 1280 m) = 160KB/partition
            at_tile = apool.tile([128, _KT, _R_BIG], f32r)

            for ci, (at_d, b_d, ct_d, R, slabs) in enumerate(chunks):
                if ci > 0:
                    # the small chunk reuses at_tile in place; a hard barrier
                    # closes any WAR/RAW scheduling window on the handover
                    tc.strict_bb_all_engine_barrier()
                atv = at_d[:].rearrange("(j p) m -> p j m", p=128)
                bv = b_d[:].rearrange("(j p) n -> p j n", p=128)
                ctv = ct_d[:].rearrange("(j p) m -> p j m", p=128)

                # load aT (k-major) into the resident tile; WAR on the
                # previous chunk's matmuls is tracked by Tile
                for j0 in range(0, _KT, 4):
                    nc.sync.dma_start(
                        out=at_tile[:, j0 : j0 + 4, 0:R],
                        in_=atv[:, j0 : j0 + 4, :],
                    )

                for n in range(_NT):
                    # B column block for this n-tile, streamed as 2 k-half
                    # blocks of 1MB each
                    bh0 = bpool.tile([128, 16, 128], f32r, name="bh", tag="bh")
                    bh1 = bpool.tile([128, 16, 128], f32r, name="bh", tag="bh")
                    # B streams ride the scalar engine's HWDGE ring so they
                    # don't queue FIFO behind the big aT loads on the sync ring
                    nc.scalar.dma_start(
                        out=bh0[:], in_=bv[:, 0:16, n * 128 : (n + 1) * 128]
                    )
                    nc.scalar.dma_start(
                        out=bh1[:], in_=bv[:, 16:32, n * 128 : (n + 1) * 128]
                    )
                    bhs = (bh0, bh1)

                    ps = [
                        psum_pool.tile([128, 512], f32, name="ps", tag="ps")
                        for _ in slabs
                    ]
                    for k in range(_KT):
                        h, kj = divmod(k, 16)
                        m0 = 0
                        for ms, w in enumerate(slabs):
                            nc.tensor.matmul(
                                ps[ms][:, 0:w],
                                bhs[h][:, kj, :],
                                at_tile[:, k, m0 : m0 + w],
                                start=(k == 0),
                                stop=(k == _KT - 1),
                            )
                            m0 += w
                    st = spool.tile([128, _R_BIG], f32, name="st", tag="st")
                    m0 = 0
                    for ms, w in enumerate(slabs):
                        nc.vector.tensor_copy(st[:, m0 : m0 + w], ps[ms][:, 0:w])
                        m0 += w
                    nc.gpsimd.dma_start(out=ctv[:, n, :], in_=st[:, 0:R])

    _split_sync_waits(nc)

    _NC_CACHE = nc
    return nc


def _shard_inputs(inputs):
    """Build the 8 per-core input maps (host-side shard + transpose)."""
    a = [np.asarray(inputs[f"a{i}"], dtype=np.float32) for i in range(8)]
    b = [np.ascontiguousarray(np.asarray(inputs[f"b{i}"], dtype=np.float32)) for i in range(8)]

    def chunk_at(g, r0, rows, width):
        c = a[g][r0 : r0 + rows]
        if rows < width:
            c = np.concatenate([c, np.zeros((width - rows, _K), np.float32)], axis=0)
        return np.ascontiguousarray(c.T)

    in_maps = []
    for c in range(8):
        gb, rb, nb = _BIG_CHUNKS[c]
        gs, rs, ns = _SMALL_CHUNKS[c]
        in_maps.append(
            {
                "at_big": chunk_at(gb, rb, nb, _R_BIG),
                "at_small": chunk_at(gs, rs, ns, _R_SMALL),
                "b_big": b[gb],
                "b_small": b[gs],
            }
        )
    return in_maps


def _gather_outputs(results):
    """Assemble full per-group outputs from per-core C^T chunks."""
    outs = [np.empty((m, _N), dtype=np.float32) for m in _M_LIST]
    for c in range(8):
        gb, rb, nb = _BIG_CHUNKS[c]
        gs, rs, ns = _SMALL_CHUNKS[c]
        outs[gb][rb : rb + nb] = np.asarray(results[c]["ct_big"]).T[:nb]
        outs[gs][rs : rs + ns] = np.asarray(results[c]["ct_small"]).T[:ns]
    return tuple(outs)


def kernel(**inputs):
    from concourse.bass_utils import run_bass_kernel_spmd

    nc = build_program()
    in_maps = _shard_inputs(inputs)
    res = run_bass_kernel_spmd(nc, in_maps, core_ids=list(range(8)))
    return _gather_outputs(res.results)


if __name__ == "__main__":
    nc = build_program()
    print("program built OK")
